# revision 1
# baseline (speedup 1.0000x reference)
"""KANLinear forward on 8 TRN2 NeuronCores.

Reference computes
    out = x @ base_w.T + base_b + spline_w @ linspace(0, 1, S)
The spline branch is batch-independent, so it folds into a single bias
vector on the host. The device kernel is a data-parallel matmul: each
core computes a [2048, 1024] batch shard as out.T tiles ([out-feature
partitions, batch free dim]) so the per-feature bias is a per-partition
scalar add fused into the PSUM->SBUF eviction.

Inputs are pre-tiled on the host into the exact SBUF layouts so every
DMA is a contiguous >=2KB-per-partition-line transfer:
  x  -> [NB, 128, KO, 512]   (nb b-tile, ki partition, ko k-subtile, b col)
  w  -> [MO, 128, KO, 128]   (mo o-tile, ki partition, ko k-subtile, m col)
  out <- [NB, 128, MO, 512]  (nb, o-partition, mo o-tile, b col)
Matmuls run in float32r (TF32-like, 1 row/cycle at N=512) with fp32 PSUM
accumulation.

DMA scheduling: each descriptor ring processes its DMAs serially at
roughly bytes/416GB/s + ~2us completion receipt, so transfers are spread
across the three independent rings (SP HWDGE via nc.sync, ACT HWDGE via
nc.scalar, SWDGE via nc.gpsimd) in PE consumption order, with small
first chunks (early PE start) and small final output chunks (short
tail).
"""

import numpy as np

import concourse.bass as bass  # noqa: F401
import concourse.mybir as mybir
import concourse.tile as tile
from concourse import bacc
from concourse.bass_utils import run_bass_kernel_spmd

B, IN, OUT = 16384, 1024, 1024
N_CORES = 8
BS = B // N_CORES  # 2048 batch rows per core
P = 128  # SBUF partitions
KO = IN // P  # 8 k-subtiles of the contraction dim
MO = OUT // P  # 8 out-feature tiles (psum partition dim)
NB_TILE = 512  # matmul free dim = one fp32 PSUM bank
NB = BS // NB_TILE  # 4 batch tiles per core

_CACHE = {}


def _build_nc():
    f32 = mybir.dt.float32
    f32r = mybir.dt.float32r

    nc = bacc.Bacc("TRN2", target_bir_lowering=False)
    x_d = nc.dram_tensor("x_t", [NB, P, KO, NB_TILE], f32r, kind="ExternalInput")
    w_d = nc.dram_tensor("w_t", [MO, P, KO, P], f32r, kind="ExternalInput")
    b_d = nc.dram_tensor("bias_t", [P, MO], f32, kind="ExternalInput")
    o_d = nc.dram_tensor("out_t", [NB, P, MO, NB_TILE], f32, kind="ExternalOutput")

    with tile.TileContext(nc) as tc:
        with (
            tc.tile_pool(name="wp", bufs=1) as wp,
            tc.tile_pool(name="xp", bufs=1) as xp,
            tc.tile_pool(name="cp", bufs=1) as cp,
            tc.tile_pool(name="op", bufs=1) as op,
            tc.tile_pool(name="ps", bufs=4, space="PSUM") as ps,
        ):
            # bias rides SWDGE (idle until outputs start)
            bias_sb = cp.tile([P, MO], f32)
            nc.gpsimd.dma_start(bias_sb[:], b_d[:])

            w_sb = [None] * MO
            x_parts = [[] for _ in range(NB)]

            def load_w(mos, engine):
                t = wp.tile([P, len(mos), KO, P], f32r, tag=f"w{mos[0]}")
                engine.dma_start(t[:], w_d[mos[0] : mos[0] + len(mos)].rearrange(
                    "a ki ko m -> ki a ko m"
                ))
                for i, mo in enumerate(mos):
                    w_sb[mo] = t[:, i]

            def load_x(nb, k0, kn, engine):
                t = xp.tile([P, kn, NB_TILE], f32r, tag=f"x{nb}_{k0}")
                engine.dma_start(t[:], x_d[nb, :, k0 : k0 + kn])
                x_parts[nb].append((k0, kn, t))

            # ACT ring: weights in PE consumption order — singles first so
            # delivery keeps pace with PE (~1 mo-tile per 2us) during nb0.
            load_w([0], nc.scalar)
            load_w([1], nc.scalar)
            load_w([2], nc.scalar)
            load_w([3], nc.scalar)
            load_w([4, 5], nc.scalar)
            load_w([6, 7], nc.scalar)
            # SP ring: x0/x1 in k-halves (earlier deps), x2/x3 whole
            # (ring receipt cost dominates finer chunks).
            KH = KO // 2
            load_x(0, 0, KH, nc.sync)
            load_x(0, KH, KH, nc.sync)
            load_x(1, 0, KH, nc.sync)
            load_x(1, KH, KH, nc.sync)
            load_x(2, 0, KO, nc.sync)
            load_x(3, 0, KO, nc.sync)

            def x_slice(nb, k):
                for k0, kn, t in x_parts[nb]:
                    if k0 <= k < k0 + kn:
                        return t[:, k - k0]
                raise AssertionError

            MH = MO // 2  # output DMA chunk = half an nb stripe (1MB)
            # last nb goes out in quarters to shorten the tail
            out_engines = {
                (0, 0): nc.gpsimd,
                (0, 1): nc.gpsimd,
                (1, 0): nc.gpsimd,
                (1, 1): nc.gpsimd,
                (2, 0): nc.sync,
                (2, 1): nc.gpsimd,
            }

            for nb in range(NB):
                if nb < NB - 1:
                    ot = [op.tile([P, MH, NB_TILE], f32, tag=f"o{nb}_{h}",
                                  name=f"o{nb}_{h}")
                          for h in range(2)]
                else:
                    ot = [op.tile([P, 2, NB_TILE], f32, tag=f"o{nb}_{q}",
                                  name=f"o{nb}_{q}")
                          for q in range(4)]
                for mo in range(MO):
                    pt = ps.tile([P, NB_TILE], mybir.dt.float32)
                    for k in range(KO):
                        nc.tensor.matmul(
                            pt[:],
                            w_sb[mo][:, k],
                            x_slice(nb, k),
                            start=(k == 0),
                            stop=(k == KO - 1),
                        )
                    if nb < NB - 1:
                        h, i = divmod(mo, MH)
                        dst = ot[h][:, i]
                    else:
                        h, i = divmod(mo, 2)
                        dst = ot[h][:, i]
                    nc.vector.tensor_scalar_add(dst, pt[:], bias_sb[:, mo : mo + 1])
                    if nb < NB - 1:
                        if mo == MH - 1:
                            out_engines[(nb, 0)].dma_start(
                                o_d[nb, :, 0:MH], ot[0][:]
                            )
                        elif mo == MO - 1:
                            out_engines[(nb, 1)].dma_start(
                                o_d[nb, :, MH:MO], ot[1][:]
                            )
                    else:
                        if mo % 2 == 1:
                            q = mo // 2
                            eng = [nc.scalar, nc.gpsimd, nc.sync, nc.scalar][q]
                            eng.dma_start(
                                o_d[nb, :, q * 2 : q * 2 + 2], ot[q][:]
                            )

    nc.finalize()
    return nc


def _get_nc():
    if "nc" not in _CACHE:
        _CACHE["nc"] = _build_nc()
    return _CACHE["nc"]


def _prep_inputs(x, base_w, base_b, spline_w):
    x = np.ascontiguousarray(np.asarray(x), dtype=np.float32)
    base_w = np.ascontiguousarray(np.asarray(base_w), dtype=np.float32)
    base_b = np.ascontiguousarray(np.asarray(base_b), dtype=np.float32)
    spline_w = np.ascontiguousarray(np.asarray(spline_w), dtype=np.float32)

    s_feats = spline_w.shape[1]
    spline_input = np.linspace(0.0, 1.0, s_feats, dtype=np.float32)
    bias = (base_b + spline_w @ spline_input).astype(np.float32)  # [OUT]

    # w_dev[mo, ki, ko, m] = base_w[mo*P + m, ko*P + ki]
    w_dev = np.ascontiguousarray(
        base_w.reshape(MO, P, KO, P).transpose(0, 3, 2, 1)
    )
    # bias_dev[p, mo] = bias[mo*P + p]
    bias_dev = np.ascontiguousarray(bias.reshape(MO, P).T)

    in_maps = []
    for c in range(N_CORES):
        xs = x[c * BS : (c + 1) * BS]  # [BS, IN]
        # x_dev[nb, ki, ko, col] = xs[nb*NB_TILE + col, ko*P + ki]
        x_dev = np.ascontiguousarray(
            xs.reshape(NB, NB_TILE, KO, P).transpose(0, 3, 2, 1)
        )
        in_maps.append({"x_t": x_dev, "w_t": w_dev, "bias_t": bias_dev})
    return in_maps


def _run(inputs, trace=False, tmpdir=None):
    nc = _get_nc()
    in_maps = _prep_inputs(**inputs)
    res = run_bass_kernel_spmd(
        nc, in_maps, core_ids=list(range(N_CORES)), trace=trace, tmpdir=tmpdir
    )
    outs = []
    for c in range(N_CORES):
        arr = np.asarray(res.results[c]["out_t"])  # [NB, P, MO, NB_TILE]
        # out_core[nb*NB_TILE + col, mo*P + p] = arr[nb, p, mo, col]
        outs.append(arr.transpose(0, 3, 2, 1).reshape(BS, OUT))
    full = np.ascontiguousarray(np.concatenate(outs, axis=0), dtype=np.float32)
    return full, res


def kernel(**inputs) -> np.ndarray:
    out, _ = _run(inputs, trace=False)
    return out



# revision 6
# speedup vs baseline: 1.2500x; 1.2500x over previous
"""KANLinear forward on 8 TRN2 NeuronCores.

Reference computes
    out = x @ base_w.T + base_b + spline_w @ linspace(0, 1, S)
The spline branch is batch-independent, so it folds into a single bias
vector on the host. The device kernel is a data-parallel matmul: each
core computes a [2048, 1024] batch shard as out.T tiles ([out-feature
partitions, batch free dim]) so the per-feature bias is a per-partition
scalar add fused into the PSUM->SBUF eviction.

v2: inputs are cast to fp16 on the host. The PE runs fp16 at the same
1 row/cycle as fp32r (measured 231ns per N=512 fp32r matmul = warm
2.4GHz rate), so fp16 does not speed the matmul itself, but it halves
load traffic (12MB -> 6MB per core), letting x arrive long before it is
consumed (the f32 baseline starved: its last x shard landed at t=98us
of a 103us kernel). fp16 keeps ~4e-4 rel err (vs 2e-3 for bf16);
accumulation stays fp32 in PSUM. The moving free dim stays 512: the
matmul output must fit one 2KB PSUM bank (512 fp32) — N=1024 fails the
s3d3_mm_num_elements ISA check.

Layouts (per-partition lines contiguous >=1KB in DRAM):
  x  -> [NB, 128, KO, 512] fp16  (nb b-tile, ki partition, ko, b col)
  w  -> [MO, 128, KO, 128] fp16  (mo o-tile, ki partition, ko, m col)
  out <- [NB, 128, MO, 512] f32  (nb, o-partition, mo, b col)

DMA schedule: three rings (SP HWDGE via nc.sync, ACT HWDGE via
nc.scalar, SWDGE via nc.gpsimd). Loads are issued in PE consumption
order with a small first chunk for an early start; stores are spread
across rings in eviction-readiness order so no ring's serial chain
(bytes/436GB/s + ~2us HBM-write receipt) lands on the critical path.
The final nb's last two output columns go out as separate 0.5MB DMAs
on two different rings to shorten the tail.
"""

import numpy as np

import concourse.bass as bass  # noqa: F401
import concourse.mybir as mybir
import concourse.tile as tile
from concourse import bacc
from concourse.bass_utils import run_bass_kernel_spmd

B, IN, OUT = 16384, 1024, 1024
N_CORES = 8
BS = B // N_CORES  # 2048 batch rows per core
P = 128  # SBUF partitions
KO = IN // P  # 8 k-subtiles of the contraction dim
MO = OUT // P  # 8 out-feature tiles (psum partition dim)
NB_TILE = 512  # matmul free dim = one fp32 PSUM bank
NB = BS // NB_TILE  # 4 batch tiles per core

_CACHE = {}


def _build_nc():
    f32 = mybir.dt.float32
    f16 = mybir.dt.float16

    nc = bacc.Bacc("TRN2", target_bir_lowering=False)
    x_d = nc.dram_tensor("x_t", [NB, P, KO, NB_TILE], f16, kind="ExternalInput")
    w_d = nc.dram_tensor("w_t", [MO, P, KO, P], f16, kind="ExternalInput")
    b_d = nc.dram_tensor("bias_t", [P, MO], f32, kind="ExternalInput")
    o_d = nc.dram_tensor("out_t", [NB, P, MO, NB_TILE], f32, kind="ExternalOutput")

    with tile.TileContext(nc) as tc:
        with (
            tc.tile_pool(name="wp", bufs=1) as wp,
            tc.tile_pool(name="xp", bufs=1) as xp,
            tc.tile_pool(name="cp", bufs=1) as cp,
            tc.tile_pool(name="op", bufs=1) as op,
            tc.tile_pool(name="ps", bufs=4, space="PSUM") as ps,
        ):
            # bias rides SWDGE (idle until outputs start)
            bias_sb = cp.tile([P, MO], f32)
            nc.gpsimd.dma_start(bias_sb[:], b_d[:])

            w_sb = [None] * MO
            x_parts = [[] for _ in range(NB)]

            def load_w(mos, engine):
                t = wp.tile([P, len(mos), KO, P], f16, tag=f"w{mos[0]}")
                engine.dma_start(t[:], w_d[mos[0] : mos[0] + len(mos)].rearrange(
                    "a ki ko m -> ki a ko m"
                ))
                for i, mo in enumerate(mos):
                    w_sb[mo] = t[:, i]

            def load_x(nb, k0, kn, engine):
                t = xp.tile([P, kn, NB_TILE], f16, tag=f"x{nb}_{k0}")
                engine.dma_start(t[:], x_d[nb, :, k0 : k0 + kn])
                x_parts[nb].append((k0, kn, t))

            # ACT ring: weights in PE consumption order, small first chunk.
            load_w([0], nc.scalar)
            load_w([1], nc.scalar)
            load_w([2, 3], nc.scalar)
            load_w([4, 5, 6, 7], nc.scalar)
            # SP ring: x in consumption order, small first chunks.
            load_x(0, 0, 2, nc.sync)
            load_x(0, 2, 4, nc.sync)
            load_x(0, 6, 2, nc.sync)
            load_x(1, 0, 4, nc.sync)
            load_x(1, 4, 4, nc.sync)
            load_x(2, 0, 8, nc.sync)
            load_x(3, 0, 8, nc.sync)

            def x_slice(nb, k):
                for k0, kn, t in x_parts[nb]:
                    if k0 <= k < k0 + kn:
                        return t[:, k - k0]
                raise AssertionError

            # output chunk -> (mo list, engine). 4-mo (1MB) chunks assigned
            # round-robin across rings in eviction-readiness order; the last
            # nb's tail goes out in shrinking pieces on separate rings.
            out_plan = {
                0: [((0, 1, 2, 3), nc.gpsimd), ((4, 5, 6, 7), nc.scalar)],
                1: [((0, 1, 2, 3), nc.sync), ((4, 5, 6, 7), nc.gpsimd)],
                2: [((0, 1, 2, 3), nc.scalar), ((4, 5, 6, 7), nc.sync)],
                3: [((0, 1, 2, 3), nc.gpsimd), ((4, 5), nc.scalar),
                    ((6,), nc.gpsimd), ((7,), nc.sync)],
            }

            for nb in range(NB):
                chunks = []
                for ci, (mos, eng) in enumerate(out_plan[nb]):
                    t = op.tile([P, len(mos), NB_TILE], f32, tag=f"o{nb}_{ci}",
                                name=f"o{nb}_{ci}")
                    chunks.append((mos, eng, t))
                for mo in range(MO):
                    pt = ps.tile([P, NB_TILE], mybir.dt.float32)
                    for k in range(KO):
                        nc.tensor.matmul(
                            pt[:],
                            w_sb[mo][:, k],
                            x_slice(nb, k),
                            start=(k == 0),
                            stop=(k == KO - 1),
                        )
                    for mos, eng, t in chunks:
                        if mo in mos:
                            i = mos.index(mo)
                            nc.vector.tensor_scalar_add(
                                t[:, i], pt[:], bias_sb[:, mo : mo + 1]
                            )
                            if mo == mos[-1]:
                                eng.dma_start(
                                    o_d[nb, :, mos[0] : mos[-1] + 1], t[:]
                                )
                            break

    nc.finalize()
    return nc


def _get_nc():
    if "nc" not in _CACHE:
        _CACHE["nc"] = _build_nc()
    return _CACHE["nc"]


def _prep_inputs(x, base_w, base_b, spline_w):
    x = np.asarray(x, dtype=np.float32)
    base_w = np.asarray(base_w, dtype=np.float32)
    base_b = np.asarray(base_b, dtype=np.float32)
    spline_w = np.asarray(spline_w, dtype=np.float32)

    s_feats = spline_w.shape[1]
    spline_input = np.linspace(0.0, 1.0, s_feats, dtype=np.float32)
    bias = (base_b + spline_w @ spline_input).astype(np.float32)  # [OUT]

    # w_dev[mo, ki, ko, m] = base_w[mo*P + m, ko*P + ki]
    w_dev = np.ascontiguousarray(
        base_w.reshape(MO, P, KO, P).transpose(0, 3, 2, 1).astype(np.float16)
    )
    # bias_dev[p, mo] = bias[mo*P + p]
    bias_dev = np.ascontiguousarray(bias.reshape(MO, P).T)

    x16 = x.astype(np.float16)
    in_maps = []
    for c in range(N_CORES):
        xs = x16[c * BS : (c + 1) * BS]  # [BS, IN]
        # x_dev[nb, ki, ko, col] = xs[nb*NB_TILE + col, ko*P + ki]
        x_dev = np.ascontiguousarray(
            xs.reshape(NB, NB_TILE, KO, P).transpose(0, 3, 2, 1)
        )
        in_maps.append({"x_t": x_dev, "w_t": w_dev, "bias_t": bias_dev})
    return in_maps


def _run(inputs, trace=False, tmpdir=None):
    nc = _get_nc()
    in_maps = _prep_inputs(**inputs)
    res = run_bass_kernel_spmd(
        nc, in_maps, core_ids=list(range(N_CORES)), trace=trace, tmpdir=tmpdir
    )
    outs = []
    for c in range(N_CORES):
        arr = np.asarray(res.results[c]["out_t"])  # [NB, P, MO, NB_TILE]
        # out_core[nb*NB_TILE + col, mo*P + p] = arr[nb, p, mo, col]
        outs.append(arr.transpose(0, 3, 2, 1).reshape(BS, OUT))
    full = np.ascontiguousarray(np.concatenate(outs, axis=0), dtype=np.float32)
    return full, res


def kernel(**inputs) -> np.ndarray:
    out, _ = _run(inputs, trace=False)
    return out


# revision 9
# speedup vs baseline: 1.3408x; 1.0726x over previous
"""KANLinear forward on 8 TRN2 NeuronCores.

Reference computes
    out = x @ base_w.T + base_b + spline_w @ linspace(0, 1, S)
The spline branch is batch-independent, so it folds into a single bias
vector on the host. The device kernel is a data-parallel matmul: each
core computes a [2048, 1024] batch shard as out.T tiles ([out-feature
partitions, batch free dim]) so the per-feature bias is a per-partition
scalar add fused into the PSUM->SBUF eviction.

v3 (measured-trace driven):
- fp16 inputs AND outputs (host casts; PSUM accumulates fp32; rel err
  ~4e-4, gate is 2e-2). fp16 runs the PE at the same 1 row/cycle as
  fp32r (231ns/mm measured warm) but halves DMA bytes: loads 6MB,
  stores 4.2MB per core vs 21MB for the f32 baseline.
- The ~6us framework preamble means the first DMA dispatch lands at
  ~7.2us and the fabric (~435GB/s/core, shared by all queues) ramps
  after that. Loads are ordered so the bytes needed first (x0, w01)
  own the early fabric: sync ring carries x0 whole; scalar carries
  w01, w4567, x2; SWDGE carries bias, w23, x1 halves, x3.
- PE warm-up: HAM starts the PE throttled at 1.2GHz and unthrottles
  after ~3.4us of sustained activity. Eight dummy matmuls on a
  memset scratch tile run during the DMA-wait window so the real
  matmul stream starts at full 2.4GHz.
- PSUM pool uses all 8 banks (bufs=8) so psum recycling never gates
  the matmul stream (evictions lag by <=2 groups).
- Stores dispatch round-robin across the three rings in
  eviction-readiness order; the last nb's tail goes out in shrinking
  pieces on separate rings (each store pays ~2us HBM-write receipt).

Layouts (per-partition lines contiguous in DRAM):
  x  -> [NB, 128, KO, 512] fp16  (nb b-tile, ki partition, ko, b col)
  w  -> [MO, 128, KO, 128] fp16  (mo o-tile, ki partition, ko, m col)
  out <- [NB, 128, MO, 512] fp16 (nb, o-partition, mo, b col)
"""

import numpy as np

import concourse.bass as bass  # noqa: F401
import concourse.mybir as mybir
import concourse.tile as tile
from concourse import bacc
from concourse.bass_utils import run_bass_kernel_spmd

B, IN, OUT = 16384, 1024, 1024
N_CORES = 8
BS = B // N_CORES  # 2048 batch rows per core
P = 128  # SBUF partitions
KO = IN // P  # 8 k-subtiles of the contraction dim
MO = OUT // P  # 8 out-feature tiles (psum partition dim)
NB_TILE = 512  # matmul free dim = one fp32 PSUM bank
NB = BS // NB_TILE  # 4 batch tiles per core
N_WARM = 8  # dummy matmuls to unthrottle the PE before real work

_CACHE = {}


def _build_nc():
    f32 = mybir.dt.float32
    f16 = mybir.dt.float16

    nc = bacc.Bacc("TRN2", target_bir_lowering=False)
    x_d = nc.dram_tensor("x_t", [NB, P, KO, NB_TILE], f16, kind="ExternalInput")
    w_d = nc.dram_tensor("w_t", [MO, P, KO, P], f16, kind="ExternalInput")
    b_d = nc.dram_tensor("bias_t", [P, MO], f32, kind="ExternalInput")
    o_d = nc.dram_tensor("out_t", [NB, P, MO, NB_TILE], f16, kind="ExternalOutput")

    with tile.TileContext(nc) as tc:
        with (
            tc.tile_pool(name="wp", bufs=1) as wp,
            tc.tile_pool(name="xp", bufs=1) as xp,
            tc.tile_pool(name="cp", bufs=1) as cp,
            tc.tile_pool(name="op", bufs=1) as op,
            tc.tile_pool(name="ps", bufs=7, space="PSUM") as ps,
        ):
            # --- PE warm-up: memset a scratch tile, run dummy matmuls so
            # the HAM clock gate opens while the loads stream in.
            warm_sb = cp.tile([P, NB_TILE], f16)
            nc.vector.memset(warm_sb[:], 0.0)
            warm_ps = ps.tile([P, NB_TILE], f32, tag="warm", bufs=1)
            for _ in range(N_WARM):
                nc.tensor.matmul(
                    warm_ps[:], warm_sb[:, :P], warm_sb[:], start=True, stop=True
                )

            bias_sb = cp.tile([P, MO], f32)
            nc.gpsimd.dma_start(bias_sb[:], b_d[:])

            w_sb = [None] * MO
            x_parts = [[] for _ in range(NB)]

            def load_w(mos, engine):
                t = wp.tile([P, len(mos), KO, P], f16, tag=f"w{mos[0]}")
                engine.dma_start(t[:], w_d[mos[0] : mos[0] + len(mos)].rearrange(
                    "a ki ko m -> ki a ko m"
                ))
                for i, mo in enumerate(mos):
                    w_sb[mo] = t[:, i]

            def load_x(nb, k0, kn, engine):
                t = xp.tile([P, kn, NB_TILE], f16, tag=f"x{nb}_{k0}")
                engine.dma_start(t[:], x_d[nb, :, k0 : k0 + kn])
                x_parts[nb].append((k0, kn, t))

            # loads, ordered so x0+w01 own the early fabric window
            load_x(0, 0, 8, nc.sync)
            load_w([0, 1], nc.scalar)
            load_w([4, 5, 6, 7], nc.scalar)
            load_x(2, 0, 8, nc.scalar)
            load_w([2, 3], nc.gpsimd)
            load_x(1, 0, 4, nc.gpsimd)
            load_x(1, 4, 4, nc.gpsimd)
            load_x(3, 0, 8, nc.gpsimd)

            def x_slice(nb, k):
                for k0, kn, t in x_parts[nb]:
                    if k0 <= k < k0 + kn:
                        return t[:, k - k0]
                raise AssertionError

            # output chunk -> (mo list, engine), round-robin in readiness
            # order; shrinking tail pieces on separate rings.
            out_plan = {
                0: [((0, 1, 2, 3), nc.sync), ((4, 5, 6, 7), nc.gpsimd)],
                1: [((0, 1, 2, 3), nc.scalar), ((4, 5, 6, 7), nc.sync)],
                2: [((0, 1, 2, 3), nc.gpsimd), ((4, 5, 6, 7), nc.scalar)],
                3: [((0, 1, 2, 3), nc.sync), ((4, 5), nc.gpsimd),
                    ((6,), nc.scalar), ((7,), nc.sync)],
            }

            for nb in range(NB):
                chunks = []
                for ci, (mos, eng) in enumerate(out_plan[nb]):
                    t = op.tile([P, len(mos), NB_TILE], f16, tag=f"o{nb}_{ci}",
                                name=f"o{nb}_{ci}")
                    chunks.append((mos, eng, t))
                for mo in range(MO):
                    pt = ps.tile([P, NB_TILE], f32)
                    for k in range(KO):
                        nc.tensor.matmul(
                            pt[:],
                            w_sb[mo][:, k],
                            x_slice(nb, k),
                            start=(k == 0),
                            stop=(k == KO - 1),
                        )
                    for mos, eng, t in chunks:
                        if mo in mos:
                            i = mos.index(mo)
                            nc.vector.tensor_scalar_add(
                                t[:, i], pt[:], bias_sb[:, mo : mo + 1]
                            )
                            if mo == mos[-1]:
                                eng.dma_start(
                                    o_d[nb, :, mos[0] : mos[-1] + 1], t[:]
                                )
                            break

    nc.finalize()
    return nc


def _get_nc():
    if "nc" not in _CACHE:
        _CACHE["nc"] = _build_nc()
    return _CACHE["nc"]


def _prep_inputs(x, base_w, base_b, spline_w):
    x = np.asarray(x, dtype=np.float32)
    base_w = np.asarray(base_w, dtype=np.float32)
    base_b = np.asarray(base_b, dtype=np.float32)
    spline_w = np.asarray(spline_w, dtype=np.float32)

    s_feats = spline_w.shape[1]
    spline_input = np.linspace(0.0, 1.0, s_feats, dtype=np.float32)
    bias = (base_b + spline_w @ spline_input).astype(np.float32)  # [OUT]

    # w_dev[mo, ki, ko, m] = base_w[mo*P + m, ko*P + ki]
    w_dev = np.ascontiguousarray(
        base_w.reshape(MO, P, KO, P).transpose(0, 3, 2, 1).astype(np.float16)
    )
    # bias_dev[p, mo] = bias[mo*P + p]
    bias_dev = np.ascontiguousarray(bias.reshape(MO, P).T)

    x16 = x.astype(np.float16)
    in_maps = []
    for c in range(N_CORES):
        xs = x16[c * BS : (c + 1) * BS]  # [BS, IN]
        # x_dev[nb, ki, ko, col] = xs[nb*NB_TILE + col, ko*P + ki]
        x_dev = np.ascontiguousarray(
            xs.reshape(NB, NB_TILE, KO, P).transpose(0, 3, 2, 1)
        )
        in_maps.append({"x_t": x_dev, "w_t": w_dev, "bias_t": bias_dev})
    return in_maps


def _run(inputs, trace=False, tmpdir=None):
    nc = _get_nc()
    in_maps = _prep_inputs(**inputs)
    res = run_bass_kernel_spmd(
        nc, in_maps, core_ids=list(range(N_CORES)), trace=trace, tmpdir=tmpdir
    )
    outs = []
    for c in range(N_CORES):
        arr = np.asarray(res.results[c]["out_t"])  # [NB, P, MO, NB_TILE] fp16
        # out_core[nb*NB_TILE + col, mo*P + p] = arr[nb, p, mo, col]
        outs.append(
            arr.astype(np.float32).transpose(0, 3, 2, 1).reshape(BS, OUT)
        )
    full = np.ascontiguousarray(np.concatenate(outs, axis=0), dtype=np.float32)
    return full, res


def kernel(**inputs) -> np.ndarray:
    out, _ = _run(inputs, trace=False)
    return out
